# revision 46
# baseline (speedup 1.0000x reference)
"""GumbelSelector Trainium2 kernel.

Math: h = relu(s @ W1 + b1); lo = h @ W2 + b2  (2 classes)
  dec  = (argmax(lo) == 1)  ==  (z > 0)         where z = h @ (W2[:,1]-W2[:,0]) + (b2[1]-b2[0])
  prob = softmax(lo)[..., 1] ==  sigmoid(z)
  Per-row correction (LB=1): if a row of dec is all zero, activate argmax(rnoise).

Sharding: data-parallel over batch B=64 -> 8 cores x 8 rows. Weights replicated.
Host pre-transposes each core's s shard to [D=256, 32768] so the DMA loads are
fully coalesced and the contraction dim lands on SBUF partitions directly.

Structure (all matmuls float32r: 1 PE cycle/row at free>=256, vs 4 for fp32):
- the main loop is DMA-bound (~94 us floor to stream s); slab loads ride the
  sync HWDGE ring, const/prob traffic the ACT ring, and DVE-dependent
  stores the GpSimd (SWDGE) ring, so no in-order sequencer ever blocks a
  slab load behind a store's semaphore wait.
- per slab (2048 tokens): layer-1 = 8 matmuls grouped [w1a x4][w1b x4]
  (one LDWEIGHTS per stationary), one relu per 1024-token tile; layer-2 +
  one fused [1, 2048] sigmoid run TWO slabs behind, so the in-order PE
  sequencer never waits on a relu the ACT engine has not reached yet.
- prob accumulates in [1, N]-row chunks on partition 0 and is stored to
  DRAM + flushed SBUF->SBUF to row-layout pc8 as each row completes,
  hidden under the loop. dec = (prob > 0.5) is one bulk [8, N] DVE op at
  the end; the all-inactive row fix-up (argmax rnoise) is branch-skipped
  via a register compare of min(rowmax prob) vs 0.5f unless needed.
"""

import sys

if "/opt/trn_rl_repo" not in sys.path:
    sys.path.insert(0, "/opt/trn_rl_repo")

import numpy as np

import concourse.bass as bass
import concourse.mybir as mybir
import concourse.tile as tile
from concourse import bacc
from concourse.bass_utils import run_bass_kernel_spmd

B, N, D = 64, 4096, 256
HID = D // 2  # 128
NCORES = 8
BPC = B // NCORES          # batch rows per core
TOK = BPC * N              # 32768 tokens per core
SLAB = 2048                # tokens per DMA slab (1 MiB per 128-partition load)
TS = 1024                  # tokens per compute tile (2 PSUM banks)
NT = TOK // TS             # 32 compute tiles
F32 = mybir.dt.float32
F32R = mybir.dt.float32r   # 1 cycle/row on the PE (vs 4 for fp32) at free>=256

_NC = None


def _build_nc():
    nc = bacc.Bacc("TRN2", target_bir_lowering=False, debug=False)
    sT = nc.dram_tensor("sT", [D, TOK], F32R, kind="ExternalInput")
    rn = nc.dram_tensor("rn", [BPC, N], F32, kind="ExternalInput")
    w1 = nc.dram_tensor("w1", [D, HID], F32R, kind="ExternalInput")
    b1 = nc.dram_tensor("b1", [HID, 1], F32, kind="ExternalInput")
    w2d = nc.dram_tensor("w2d", [HID, 1], F32R, kind="ExternalInput")
    b2d = nc.dram_tensor("b2d", [1, 1], F32, kind="ExternalInput")
    dec = nc.dram_tensor("dec", [BPC, N], F32, kind="ExternalOutput")
    prob = nc.dram_tensor("prob", [BPC, N], F32, kind="ExternalOutput")

    AF = mybir.ActivationFunctionType
    ALU = mybir.AluOpType

    with tile.TileContext(nc) as tc:
        with (
            tc.tile_pool(name="consts", bufs=1) as consts,
            tc.tile_pool(name="io8", bufs=1) as io8,
            tc.tile_pool(name="sapool", bufs=5) as sapool,
            tc.tile_pool(name="sbpool", bufs=5) as sbpool,
            tc.tile_pool(name="hpool", bufs=6) as hpool,
            tc.tile_pool(name="ckpool", bufs=2) as ckpool,
            tc.tile_pool(name="phpool", bufs=1, space=bass.MemorySpace.PSUM) as phpool,
            tc.tile_pool(name="pzpool", bufs=1, space=bass.MemorySpace.PSUM) as pzpool,
        ):
            # tiny const loads go first (the sync HWDGE ring is FIFO: a big
            # slab load ahead of them would delay the first matmul by the
            # whole slab transfer)
            w1a = consts.tile([128, HID], F32R)
            nc.scalar.dma_start(w1a[:], w1[0:128, :])
            w1b = consts.tile([128, HID], F32R)
            nc.scalar.dma_start(w1b[:], w1[128:256, :])
            b1s = consts.tile([HID, 1], F32)
            nc.scalar.dma_start(b1s[:], b1[:])
            w2s = consts.tile([HID, 1], F32R)
            nc.scalar.dma_start(w2s[:], w2d[:])
            b2s = consts.tile([1, 1], F32)
            nc.scalar.dma_start(b2s[:], b2d[:])
            rns = io8.tile([BPC, N], F32)
            nc.scalar.dma_start(rns[:], rn[:])

            # per-row max of rnoise, computed up front (overlaps main loop)
            rmaxr = io8.tile([BPC, 1], F32)
            nc.vector.tensor_reduce(rmaxr[:], rns[:], mybir.AxisListType.X, ALU.max)

            # prob rows land here via SBUF->SBUF flushes (engines can only
            # address base partitions 0/32/64/96, so ACT can't write row c);
            # dec is derived from it in one bulk pass at the end
            pc8 = io8.tile([BPC, N], F32)

            # m16r[0, p] = max(prob) over half-row piece p, written per
            # stage2 pair on partition 0 so the end-of-kernel fix-up can be
            # branch-skipped cheaply (minimum over pieces == minimum over
            # row maxima when checked against the 0.5 threshold per row...
            # here each row is exactly two pieces, so rowmax(c) =
            # max(piece 2c, piece 2c+1); the branch needs min over rows of
            # rowmax, tested as: any row with BOTH pieces <= 0.5)
            m16r = io8.tile([1, 2 * BPC], F32)
            m8r = io8.tile([1, BPC], F32)

            state = {"chunk": None}
            pending = []  # slabs awaiting layer-2, kept 2 deep so the PE
            # never waits on a relu the ACT engine has not reached yet

            def stage2_pair(pgroup):
                # layer-2 + one fused sigmoid for a finished slab, grouped so
                # the w2s stationary loads once per slab
                (hA, tA), (hB, tB) = pgroup
                pz = pzpool.tile([1, 2 * TS], F32, name="pz")
                nc.tensor.matmul(pz[0:1, 0:512], w2s[:], hA[:, 0:512],
                                 start=True, stop=True)
                nc.tensor.matmul(pz[0:1, 512:1024], w2s[:], hA[:, 512:1024],
                                 start=True, stop=True)
                nc.tensor.matmul(pz[0:1, 1024:1536], w2s[:], hB[:, 0:512],
                                 start=True, stop=True)
                nc.tensor.matmul(pz[0:1, 1536:2048], w2s[:], hB[:, 512:1024],
                                 start=True, stop=True)
                coff = tA % N
                if coff == 0:
                    state["chunk"] = ckpool.tile([1, N], F32, name="chunk")
                chunk = state["chunk"]
                nc.scalar.activation(chunk[0:1, coff : coff + 2 * TS],
                                     pz[0:1, :], AF.Sigmoid, bias=b2s[:])
                p = tA // (2 * TS)
                nc.vector.tensor_reduce(m16r[0:1, p : p + 1],
                                        chunk[0:1, coff : coff + 2 * TS],
                                        mybir.AxisListType.X, ALU.max)
                if coff + 2 * TS == N:
                    c = tA // N
                    # prob store rides the ACT ring (data ready the moment
                    # the sequencer reaches it — no stall); other stores go
                    # via the idle GpSimd (SWDGE) ring so no in-order
                    # sequencer ever waits on a store
                    nc.scalar.dma_start(prob[c : c + 1, :], chunk[:])
                    nc.gpsimd.dma_start(pc8[c : c + 1, :], chunk[:])

            for si in range(TOK // SLAB):
                off = si * SLAB
                sa = sapool.tile([128, SLAB], F32R)
                sb = sbpool.tile([128, SLAB], F32R)
                if si == 0:
                    # split the first slab into quarter loads so the first
                    # matmul's operand lands ~4x sooner (warmup)
                    q = SLAB // 4
                    for k in range(4):
                        nc.sync.dma_start(sa[:, k * q : (k + 1) * q],
                                          sT[0:128, off + k * q : off + (k + 1) * q])
                    for k in range(4):
                        nc.sync.dma_start(sb[:, k * q : (k + 1) * q],
                                          sT[128:256, off + k * q : off + (k + 1) * q])
                else:
                    nc.sync.dma_start(sa[:], sT[0:128, off : off + SLAB])
                    nc.sync.dma_start(sb[:], sT[128:256, off : off + SLAB])
                # layer-2 of the slab before last first: its deps are two
                # slabs old, so the in-order PE/ACT sequencers never stall on
                # a cross-engine round-trip
                if len(pending) >= 2:
                    stage2_pair(pending.pop(0))
                # layer-1 for both tiles of this slab, each stationary loaded
                # once: [w1a x4][w1b x4]
                phA = phpool.tile([128, TS], F32, name="phA")
                phB = phpool.tile([128, TS], F32, name="phB")
                nc.tensor.matmul(phA[:, 0:512], w1a[:], sa[:, 0:512],
                                 start=True, stop=False)
                nc.tensor.matmul(phA[:, 512:1024], w1a[:], sa[:, 512:1024],
                                 start=True, stop=False)
                nc.tensor.matmul(phB[:, 0:512], w1a[:], sa[:, 1024:1536],
                                 start=True, stop=False)
                nc.tensor.matmul(phB[:, 512:1024], w1a[:], sa[:, 1536:2048],
                                 start=True, stop=False)
                nc.tensor.matmul(phA[:, 0:512], w1b[:], sb[:, 0:512],
                                 start=False, stop=True)
                nc.tensor.matmul(phA[:, 512:1024], w1b[:], sb[:, 512:1024],
                                 start=False, stop=True)
                nc.tensor.matmul(phB[:, 0:512], w1b[:], sb[:, 1024:1536],
                                 start=False, stop=True)
                nc.tensor.matmul(phB[:, 512:1024], w1b[:], sb[:, 1536:2048],
                                 start=False, stop=True)
                hA = hpool.tile([128, TS], F32R, name="hA")
                nc.scalar.activation(hA[:], phA[:], AF.Relu, bias=b1s[:])
                hB = hpool.tile([128, TS], F32R, name="hB")
                nc.scalar.activation(hB[:], phB[:], AF.Relu, bias=b1s[:])
                pending.append(((hA, off), (hB, off + TS)))
            while pending:
                stage2_pair(pending.pop(0))

            # Rare fix-up: only if some row is all-inactive (rowmax prob <=
            # 0.5). Positive fp32 bit patterns order like the floats, so the
            # raw-bits register compare against 0.5f is exact. The dec rows
            # already stored speculatively are re-stored corrected via a
            # conditional DMA that is skipped in the common case.
            # rowmax(c) = max of the row's two pieces, then min over rows
            nc.vector.tensor_max(m8r[:], m16r[0:1, 0 : 2 * BPC : 2],
                                 m16r[0:1, 1 : 2 * BPC : 2])
            mmin = io8.tile([1, 1], F32)
            nc.vector.tensor_reduce(mmin[:], m8r[:], mybir.AxisListType.X,
                                    ALU.min)
            nc.vector.tensor_scalar(pc8[:], pc8[:], 0.5, None, ALU.is_gt)
            v = nc.vector.value_load(mmin[0:1, 0:1].bitcast(mybir.dt.int32))
            with tc.If(v <= 0x3F000000):
                rmaxd = io8.tile([BPC, 1], F32)
                nc.vector.tensor_reduce(rmaxd[:], pc8[:],
                                        mybir.AxisListType.X, ALU.max)
                need = io8.tile([BPC, 1], F32)
                nc.vector.tensor_scalar(need[:], rmaxd[:], 0.0, None,
                                        ALU.is_equal)
                # tensor-tensor compare with a broadcast AP: the scalar-
                # operand compare path quantizes the scalar and can match
                # several near-max values; the tensor path compares exactly
                nc.vector.tensor_tensor(rns[:], rns[:],
                                        rmaxr[:].broadcast_to([BPC, N]),
                                        ALU.is_equal)
                nc.vector.tensor_scalar(rns[:], rns[:], need[:], None,
                                        ALU.mult)
                nc.vector.tensor_max(pc8[:], pc8[:], rns[:])
            nc.sync.dma_start(dec[:], pc8[:])

    nc.compile()
    return nc


def _get_nc():
    global _NC
    if _NC is None:
        _NC = _build_nc()
    return _NC


def _make_in_maps(s, W1, b1, W2, b2, rnoise):
    s = np.ascontiguousarray(s, dtype=np.float32)
    w1 = np.ascontiguousarray(W1, dtype=np.float32)
    b1c = np.ascontiguousarray(b1, dtype=np.float32).reshape(HID, 1)
    w2dc = np.ascontiguousarray(W2[:, 1] - W2[:, 0], dtype=np.float32).reshape(HID, 1)
    b2dc = np.array([[b2[1] - b2[0]]], dtype=np.float32)
    rn = np.ascontiguousarray(rnoise, dtype=np.float32)

    # [NCORES, D, TOK] with the contraction dim outer -> coalesced loads
    sT = np.ascontiguousarray(
        s.reshape(NCORES, TOK, D).transpose(0, 2, 1)
    )
    return [
        {
            "sT": sT[c],
            "rn": rn.reshape(NCORES, BPC, N)[c],
            "w1": w1,
            "b1": b1c,
            "w2d": w2dc,
            "b2d": b2dc,
        }
        for c in range(NCORES)
    ]


def run(s, W1, b1, W2, b2, rnoise, trace=False):
    nc = _get_nc()
    in_maps = _make_in_maps(s, W1, b1, W2, b2, rnoise)
    res = run_bass_kernel_spmd(nc, in_maps, list(range(NCORES)), trace=trace)
    dec = np.concatenate([r["dec"] for r in res.results], axis=0)
    prob = np.concatenate([r["prob"] for r in res.results], axis=0)
    return (dec, prob), res


def kernel(s, W1, b1, W2, b2, rnoise):
    (dec, prob), _ = run(s, W1, b1, W2, b2, rnoise)
    return dec, prob
